# revision 15
# baseline (speedup 1.0000x reference)
"""Trainium2 Bass kernel for nn_CrossAttention_24438363914471.

Cross-attention module: B=8, C=512, H=W=48 (N=2304 tokens per batch image).
Reference computation per batch b:
    q = lf^T Wq^T + bq ; k = gf^T Wk^T + bk ; v = gf^T Wv^T + bv
    attn = softmax(q k^T) ; out = attn v ; out = out Wo^T + bo
    result = lf + out^T ; output = Wconv . result + bconv      # 1x1 conv C->1

Because the final 1x1 conv collapses all C channels into one scalar per pixel,
nearly everything folds (computed host-side, weights only — no activations):
    A      = Wq^T Wk                 (then S = lf^T A gf + rowterm + q-only terms)
    rowterm= (Wk^T bq)^T gf          (k-dependent softmax bias; q-only terms cancel)
    weff   = Wo^T Wconv^T            ->  wv = Wv^T weff  (so  Wconv.(Wo attn_v) =
                                          sum_k p_k (wv.gf_k) / sum_k p_k + consts)
    out[q] = Wconv.lf_q + num[q]/den[q] + (weff.bv + Wconv.bo + bconv)

Device work per core (1 batch element, data-parallel over B across 8 cores):
    U  = A gf                                  [512,2304]   96 matmuls
    T0 = U^T lf  (attention logits^T)          [2304,2304] 432 matmuls
    P  = exp(T0 + rowterm - CM)   (ACT engine, constant shift CM: softmax is
                                   shift-invariant; CM only prevents overflow)
    [num;den] = [vw|1]^T P                     [2,2304]    108 matmuls
plus tiny vector matmuls (rowterm, vw.gf, Wconv.lf) and an O(N) epilogue.
Logit-path matmuls run in fp16 (fp32 lowers to 2 slow LOW_HIGH passes on the
PE; fp16 is single-pass at N/2.4GHz), exp/num-den in bf16 (fp16 would
overflow at exp values up to e^37). num/den accumulate in fp32 PSUM.
FP8 was evaluated numerically and rejected: logit std is ~22 so the softmax is
extremely peaked; e4m3 rounding of lf/U adds ~0.5 abs logit noise which
reshuffles the top keys (rel err 0.4-0.8 vs the 2e-2 gate).

v2 perf structure (vs the 141.8us v1):
  * 24 dummy warm-up matmuls on a memset scratch tile run during the initial
    DMA wait so the PE's HAM clock-gate is already at 8/8 when real matmuls
    start (v1 ran its first ~9us of matmuls at 1.2GHz).
  * inputs arrive as one DMA per chunk-aligned 512-col slice covering all 4
    channel tiles (11 calls on 3 queues, chunk-priority order).
  * per chunk: 1b (rowterm/vwgf) and 1c (convlf) run CONCURRENTLY on the PE
    via 2-way column tiling (their stationaries are 2/1 columns wide), then
    1a (U).  U PSUM->SBUF copies moved from ACT to DVE to keep ACT free.
  * num/den matmuls are 4-way column-tiled (stationary is [128,2]): 4 PE
    column groups each contract ~5 of the 18 k-tiles concurrently into
    disjoint PSUM partition pairs {0,1},{32,33},{64,65},{96,97}; wall cost
    drops from 18 to ~5 moving passes per chunk.
  * the epilogue stays in ROW space (q on the free axis): the 4 column-group
    partials are pair-added, divided and added to convlf as [1..4,w] rows,
    then stored straight to out[q0:q0+w] (contiguous DMA) per chunk.  Only
    rowterm/vwgf (which feed per-PARTITION consumers: the exp bias and the
    num/den stationary) round-trip through DRAM for the k-major -> [128,18]
    transpose; DMA hardware cannot express that transpose SBUF->SBUF (3-dim
    AP limit + contiguous-final-dim rule), and convlf/out never need it.
"""

import numpy as np
from contextlib import ExitStack

import concourse.bass as bass
import concourse.tile as tile
from concourse import bacc, mybir
from concourse.bass_utils import run_bass_kernel_spmd
from concourse.tile import add_dep_helper

F32 = mybir.dt.float32
F16 = mybir.dt.float16
BF16 = mybir.dt.bfloat16
P = 128                 # partitions
C = 512                 # channels
HW = 2304               # tokens per batch (48*48)
NCT = C // P            # 4 channel tiles
NKT = HW // P           # 18 key tiles
NCORES = 8
CHUNKS = [(0, 256), (256, 512), (768, 512), (1280, 512), (1792, 256), (2048, 256)]
CM = 105.0              # constant softmax shift (true row maxes are ~57..142)
NWARM = 24              # warm-up matmuls (N=128 each, ~2.6us of PE activity)

_EXP = mybir.ActivationFunctionType.Exp
_ADD = mybir.AluOpType.add


def _build_program(const_add: float) -> bacc.Bacc:
    nc = bacc.Bacc("TRN2", target_bir_lowering=False, debug=False)

    lf_d = nc.dram_tensor("lf", (NCT, P, HW), F16, kind="ExternalInput").ap()
    gf_d = nc.dram_tensor("gf", (NCT, P, HW), F16, kind="ExternalInput").ap()
    at_d = nc.dram_tensor("at", (P, NCT, NCT, P), F16, kind="ExternalInput").ap()
    vecs_d = nc.dram_tensor("vecs", (P, NCT, 3), F16, kind="ExternalInput").ap()
    vtmp = nc.dram_tensor("vtmp", (2, HW), F32, kind="Internal").ap()
    dtmp = nc.dram_tensor("dtmp", (1, HW), F32, kind="Internal").ap()
    out_d = nc.dram_tensor("out", (HW,), F32, kind="ExternalOutput").ap()

    with tile.TileContext(nc) as tc, ExitStack() as ctx:
        big = ctx.enter_context(tc.tile_pool(name="big", bufs=1))
        small = ctx.enter_context(tc.tile_pool(name="small", bufs=1))
        ppool = ctx.enter_context(tc.tile_pool(name="pp", bufs=20))
        stg = ctx.enter_context(tc.tile_pool(name="stg", bufs=2))
        rows = ctx.enter_context(tc.tile_pool(name="rows", bufs=3))
        psA = ctx.enter_context(tc.tile_pool(name="psA", bufs=5, space="PSUM"))
        psB = ctx.enter_context(tc.tile_pool(name="psB", bufs=2, space="PSUM"))
        psV = ctx.enter_context(tc.tile_pool(name="psV", bufs=1, space="PSUM"))

        gf_sb = big.tile([P, NCT, HW], F16, tag="gf")
        lf_sb = big.tile([P, NCT, HW], F16, tag="lf")
        u_sb = big.tile([P, NCT, HW], F16, tag="u")
        at_sb = small.tile([P, NCT, NCT, P], F16, tag="at")
        vecs_sb = small.tile([P, NCT, 3], F16, tag="vecs")
        wtile = small.tile([P, P], F16, tag="warm")
        clf_row = small.tile([1, HW], F32, tag="clf")    # convlf + nothing (row space)

        r_sb = small.tile([P, NKT], F32, tag="r")
        vwg32 = small.tile([P, NKT], F32, tag="vwg")
        biasR = small.tile([P, NKT], F32, tag="biasR")
        vwones = small.tile([P, 2, NKT], BF16, tag="vwones")

        # ---- warm-up: memset a scratch tile, then NWARM dummy matmuls so the
        # PE HAM clock-gate reaches 8/8 (2.4GHz) during the initial DMA wait.
        nc.gpsimd.memset(wtile, 0.015625)
        wps = psB.tile([P, P], F32, tag="nd")
        for _ in range(NWARM):
            nc.tensor.matmul(wps, wtile, wtile, start=True, stop=True)

        nc.vector.memset(vwones[:, 1:2, :], 1.0)

        # ---- input DMAs: one call per chunk-aligned slice covering all 4
        # channel tiles; spread across the 3 DMA-capable engines in chunk
        # priority order.  (each dma_start costs ~650ns on the issuing engine)
        def load(eng, dst_sb, src_d, h0, hw_):
            eng.dma_start(
                dst_sb[:, :, h0 : h0 + hw_],
                src_d[:, :, h0 : h0 + hw_].rearrange("t p c -> p t c"),
            )

        nc.gpsimd.dma_start(vecs_sb, vecs_d)
        nc.gpsimd.dma_start(at_sb[:, 0:2], at_d[:, 0:2])
        nc.gpsimd.dma_start(at_sb[:, 2:4], at_d[:, 2:4])
        load(nc.sync, gf_sb, gf_d, 0, 256)
        load(nc.scalar, lf_sb, lf_d, 0, 256)
        load(nc.sync, gf_sb, gf_d, 256, 512)
        load(nc.scalar, lf_sb, lf_d, 256, 512)
        load(nc.sync, gf_sb, gf_d, 768, 512)
        load(nc.scalar, lf_sb, lf_d, 768, 512)
        load(nc.sync, gf_sb, gf_d, 1280, 512)
        load(nc.scalar, lf_sb, lf_d, 1280, 512)
        load(nc.sync, gf_sb, gf_d, 1792, 512)
        load(nc.scalar, lf_sb, lf_d, 1792, 512)

        # ---- phase 1, chunked: 1b (rowterm/vwgf) || 1c (convlf) via 2-way
        # column tiling, then 1a (U = A gf).
        vec_stores = []
        for ci_, (q0, w) in enumerate(CHUNKS):
            ps1 = psV.tile([34, w], F32, tag="vecps")
            for ci in range(NCT):
                nc.tensor.matmul(
                    ps1[0:2, :],
                    vecs_sb[:, ci, 0:2],
                    gf_sb[:, ci, q0 : q0 + w],
                    start=(ci == 0),
                    stop=(ci == NCT - 1),
                    tile_position=(0, 0),
                    skip_group_check=True,
                )
                nc.tensor.matmul(
                    ps1[32:33, :],
                    vecs_sb[:, ci, 2:3],
                    lf_sb[:, ci, q0 : q0 + w],
                    start=(ci == 0),
                    stop=(ci == NCT - 1),
                    tile_position=(0, 32),
                    skip_group_check=True,
                )
            st = stg.tile([2, w], F32, tag="vstage")
            nc.scalar.copy(st, ps1[0:2, :])
            nc.scalar.copy(clf_row[0:1, q0 : q0 + w], ps1[32:33, :])
            eng = nc.sync if ci_ % 2 == 0 else nc.gpsimd
            vec_stores.append(eng.dma_start(vtmp[:, q0 : q0 + w], st))

            for co in range(NCT):
                ps = psA.tile([P, w], F32, tag="ps")
                for ci in range(NCT):
                    nc.tensor.matmul(
                        ps,
                        at_sb[:, co, ci, :],
                        gf_sb[:, ci, q0 : q0 + w],
                        start=(ci == 0),
                        stop=(ci == NCT - 1),
                    )
                nc.vector.tensor_copy(u_sb[:, co, q0 : q0 + w], ps)

        # ---- reshape rowterm / vw.gf into [128,18] partition-major tiles
        # (q = t*128 + p bijection) and build the per-key exp bias.
        ld = nc.sync.dma_start(r_sb, vtmp[0].rearrange("(t p) -> p t", p=P))
        for s in vec_stores:
            add_dep_helper(ld.ins, s.ins, reason="dram raw rowterm")
        ld = nc.gpsimd.dma_start(vwg32, vtmp[1].rearrange("(t p) -> p t", p=P))
        for s in vec_stores:
            add_dep_helper(ld.ins, s.ins, reason="dram raw vwgf")
        nc.vector.tensor_scalar_add(biasR, r_sb, -CM)
        nc.vector.tensor_copy(vwones[:, 0:1, :], vwg32)

        # ---- phase 2 per chunk: logits + exp for all 18 k-tiles, then the
        # num/den contraction 4-way column-tiled (bf16 after fp16 keeps PE
        # dtype switches at 2 per chunk).  Group g takes k-tiles g, g+4, ...
        # into PSUM partitions {32g, 32g+1}.  Division + convlf add happen in
        # row space and the result DMAs contiguously to out[q0:q0+w].
        for ci_, (q0, w) in enumerate(CHUNKS):
            pexps = []
            for kt in range(NKT):
                t0 = psA.tile([P, w], F32, tag="ps")
                for ct in range(NCT):
                    nc.tensor.matmul(
                        t0,
                        u_sb[:, ct, kt * P : (kt + 1) * P],
                        lf_sb[:, ct, q0 : q0 + w],
                        start=(ct == 0),
                        stop=(ct == NCT - 1),
                    )
                pexp = ppool.tile([P, w], BF16, tag="pexp")
                nc.scalar.activation(
                    pexp, t0, _EXP, bias=biasR[:, kt : kt + 1], scale=1.0
                )
                pexps.append(pexp)

            nd = psB.tile([P, w], F32, tag="nd")
            nkts = [len(range(g, NKT, 4)) for g in range(4)]
            done = [0, 0, 0, 0]
            for r in range(max(nkts)):
                for g in range(4):
                    kt = g + 4 * r
                    if kt >= NKT:
                        continue
                    done[g] += 1
                    nc.tensor.matmul(
                        nd[32 * g : 32 * g + 2, :],
                        vwones[:, :, kt : kt + 1],
                        pexps[kt],
                        start=(done[g] == 1),
                        stop=(done[g] == nkts[g]),
                        tile_position=(0, 32 * g),
                        skip_group_check=True,
                    )

            # combine the 4 column-group partials and divide, all in row space
            # (tensor_tensor may read at most ONE operand from PSUM)
            st4a = rows.tile([2, w], F32, tag="st4a")
            st4b = rows.tile([2, w], F32, tag="st4b")
            nc.vector.tensor_copy(st4a, nd[0:2, :])
            nc.vector.tensor_add(st4a, st4a, nd[32:34, :])
            nc.vector.tensor_copy(st4b, nd[64:66, :])
            nc.vector.tensor_add(st4b, st4b, nd[96:98, :])
            nd2 = rows.tile([2, w], F32, tag="nd2")
            nc.vector.tensor_add(nd2, st4a, st4b)
            # engines need 32-aligned partition bases; DMA moves the den row
            # (partition 1) down to partition 0 of its own tile (DRAM bounce:
            # the runtime rejects SBUF->SBUF descriptors).
            den0 = rows.tile([1, w], F32, tag="den0")
            deng = nc.sync if ci_ % 2 == 0 else nc.gpsimd
            dst = deng.dma_start(dtmp[0:1, q0 : q0 + w], nd2[1:2, :])
            dld = deng.dma_start(den0, dtmp[0:1, q0 : q0 + w])
            add_dep_helper(dld.ins, dst.ins, reason="dram raw den")
            rec = rows.tile([1, w], F32, tag="rec")
            nc.vector.reciprocal(rec, den0)
            res = rows.tile([1, w], F32, tag="res")
            nc.vector.tensor_mul(res, nd2[0:1, :], rec)
            fin = rows.tile([1, w], F32, tag="fin")
            nc.vector.scalar_tensor_tensor(
                fin, res, float(const_add), clf_row[0:1, q0 : q0 + w],
                op0=_ADD, op1=_ADD,
            )
            eng = nc.sync if ci_ % 2 == 0 else nc.gpsimd
            # NOTE: fin (2-D [1,w]) not fin[0] — a fully-1-D SBUF source AP
            # makes the runtime reject the NEFF at load time.
            eng.dma_start(out_d[q0 : q0 + w], fin)

    nc.compile()
    return nc


_CACHE: dict[bytes, bacc.Bacc] = {}


def _fold(inputs):
    f64 = np.float64
    Wq, bq = inputs["Wq"].astype(f64), inputs["bq"].astype(f64)
    Wk, bk = inputs["Wk"].astype(f64), inputs["bk"].astype(f64)
    Wv, bv = inputs["Wv"].astype(f64), inputs["bv"].astype(f64)
    Wo, bo = inputs["Wo"].astype(f64), inputs["bo"].astype(f64)
    Wconv, bconv = inputs["Wconv"].astype(f64), inputs["bconv"].astype(f64)

    A = Wq.T @ Wk                       # S0 = lf^T A gf
    AT = np.ascontiguousarray(
        A.T.astype(np.float16).reshape(NCT, P, NCT, P).transpose(1, 2, 0, 3)
    )
    wkb = Wk.T @ bq                     # rowterm = wkb^T gf
    weff = Wo.T @ Wconv[0]
    wv = Wv.T @ weff
    vecs = np.stack(
        [wkb.astype(np.float32), wv.astype(np.float32), inputs["Wconv"][0]], axis=1
    )                                   # [C, 3]
    vecs = np.ascontiguousarray(
        vecs.astype(np.float16).reshape(NCT, P, 3).transpose(1, 0, 2)
    )
    const_add = float(weff @ bv + Wconv[0] @ bo + bconv[0])
    return AT, vecs, const_add


def _prepare_in_maps(inputs):
    AT, vecs, const_add = _fold(inputs)
    lf = np.ascontiguousarray(inputs["local_feat"].astype(np.float16)).reshape(
        NCORES, NCT, P, HW
    )
    gf = np.ascontiguousarray(inputs["global_feat"].astype(np.float16)).reshape(
        NCORES, NCT, P, HW
    )
    in_maps = [
        {"lf": lf[b], "gf": gf[b], "at": AT, "vecs": vecs} for b in range(NCORES)
    ]
    return in_maps, const_add


def run(inputs, trace: bool = False, **kwargs):
    """Run on hardware; returns (output [8,1,48,48], BassKernelResults)."""
    in_maps, const_add = _prepare_in_maps(inputs)
    key = np.float32(const_add).tobytes()
    if key not in _CACHE:
        _CACHE[key] = _build_program(const_add)
    nc = _CACHE[key]
    res = run_bass_kernel_spmd(
        nc, in_maps, core_ids=list(range(NCORES)), trace=trace, **kwargs
    )
    out = np.stack([res.results[b]["out"] for b in range(NCORES)], axis=0)
    return out.reshape(NCORES, 1, 48, 48).astype(np.float32), res


def kernel(**inputs) -> np.ndarray:
    out, _ = run(inputs)
    return out


# revision 16
# speedup vs baseline: 1.1161x; 1.1161x over previous
"""Trainium2 Bass kernel for nn_CrossAttention_24438363914471.

Cross-attention module: B=8, C=512, H=W=48 (N=2304 tokens per batch image).
Reference computation per batch b:
    q = lf^T Wq^T + bq ; k = gf^T Wk^T + bk ; v = gf^T Wv^T + bv
    attn = softmax(q k^T) ; out = attn v ; out = out Wo^T + bo
    result = lf + out^T ; output = Wconv . result + bconv      # 1x1 conv C->1

Because the final 1x1 conv collapses all C channels into one scalar per pixel,
nearly everything folds (computed host-side, weights only — no activations):
    A      = Wq^T Wk                 (then S = lf^T A gf + rowterm + q-only terms)
    rowterm= (Wk^T bq)^T gf          (k-dependent softmax bias; q-only terms cancel)
    weff   = Wo^T Wconv^T            ->  wv = Wv^T weff  (so  Wconv.(Wo attn_v) =
                                          sum_k p_k (wv.gf_k) / sum_k p_k + consts)
    out[q] = Wconv.lf_q + num[q]/den[q] + (weff.bv + Wconv.bo + bconv)

Device work per core (1 batch element, data-parallel over B across 8 cores):
    U  = A gf                                  [512,2304]   96 matmuls
    T0 = U^T lf  (attention logits^T)          [2304,2304] 432 matmuls
    P  = exp(T0 + rowterm - CM)   (ACT engine, constant shift CM: softmax is
                                   shift-invariant; CM only prevents overflow)
    [num;den] = [vw|1]^T P                     [2,2304]    108 matmuls
plus tiny vector matmuls (rowterm, vw.gf, Wconv.lf) and an O(N) epilogue.
Logit-path matmuls run in fp16 (fp32 lowers to 2 slow LOW_HIGH passes on the
PE; fp16 is single-pass at N/2.4GHz), exp/num-den in bf16 (fp16 would
overflow at exp values up to e^37). num/den accumulate in fp32 PSUM.
FP8 was evaluated numerically and rejected: logit std is ~22 so the softmax is
extremely peaked; e4m3 rounding of lf/U adds ~0.5 abs logit noise which
reshuffles the top keys (rel err 0.4-0.8 vs the 2e-2 gate).
PE column-tiling (tile_position) of the [128,2]-stationary num/den matmuls was
tried and measured ZERO concurrency (each col-group still pays a full moving
pass when the moving operands differ) — reverted.

Perf structure (vs the 141.8us v1 baseline):
  * 32 dummy warm-up matmuls (~3.4us = one full HAM window) on a memset
    scratch tile run during the initial DMA wait so the PE clock-gate is at
    8/8 (2.4GHz) when real matmuls start (v1 ran ~9us of matmuls at 1.2GHz).
  * per chunk, 1b (rowterm/vw.gf) runs BEFORE 1a so the last rowterm store
    lands early; the [2,HW]->[128,18] transpose round-trip through DRAM then
    overlaps the tail of phase 1 instead of stalling phase 2's first exp.
  * the epilogue stays in ROW space (q on the free axis): num/den partials
    are divided and added to convlf as [1..2,w] rows per chunk, then stored
    straight to out[q0:q0+w] (contiguous DMA) per chunk.  The tail after the
    last matmul is only the last (smallest) chunk's epilogue, not a full
    [2,2304] DRAM round-trip + transpose + gather (v1 tail was ~6us).
  * convlf (1c) output never leaves row space (it lands in clf_row and is
    consumed by the row-space epilogue) — no reshape round-trip for it.
"""

import numpy as np
from contextlib import ExitStack

import concourse.bass as bass
import concourse.tile as tile
from concourse import bacc, mybir
from concourse.bass_utils import run_bass_kernel_spmd
from concourse.tile import add_dep_helper

F32 = mybir.dt.float32
F16 = mybir.dt.float16
BF16 = mybir.dt.bfloat16
P = 128                 # partitions
C = 512                 # channels
HW = 2304               # tokens per batch (48*48)
NCT = C // P            # 4 channel tiles
NKT = HW // P           # 18 key tiles
NCORES = 8
CHUNKS = [(0, 256), (256, 512), (768, 512), (1280, 512), (1792, 256), (2048, 256)]
CM = 105.0              # constant softmax shift (true row maxes are ~57..142)
NWARM = 32              # warm-up matmuls (N=128 each, ~3.4us = one HAM window)

_EXP = mybir.ActivationFunctionType.Exp
_ADD = mybir.AluOpType.add


def _build_program(const_add: float) -> bacc.Bacc:
    nc = bacc.Bacc("TRN2", target_bir_lowering=False, debug=False)

    lf_d = nc.dram_tensor("lf", (NCT, P, HW), F16, kind="ExternalInput").ap()
    gf_d = nc.dram_tensor("gf", (NCT, P, HW), F16, kind="ExternalInput").ap()
    at_d = nc.dram_tensor("at", (P, NCT, NCT, P), F16, kind="ExternalInput").ap()
    vecs_d = nc.dram_tensor("vecs", (P, NCT, 3), F16, kind="ExternalInput").ap()
    vtmp = nc.dram_tensor("vtmp", (2, HW), F32, kind="Internal").ap()
    dtmp = nc.dram_tensor("dtmp", (1, HW), F32, kind="Internal").ap()
    out_d = nc.dram_tensor("out", (HW,), F32, kind="ExternalOutput").ap()

    with tile.TileContext(nc) as tc, ExitStack() as ctx:
        big = ctx.enter_context(tc.tile_pool(name="big", bufs=1))
        small = ctx.enter_context(tc.tile_pool(name="small", bufs=1))
        ppool = ctx.enter_context(tc.tile_pool(name="pp", bufs=20))
        stg = ctx.enter_context(tc.tile_pool(name="stg", bufs=2))
        rows = ctx.enter_context(tc.tile_pool(name="rows", bufs=3))
        psA = ctx.enter_context(tc.tile_pool(name="psA", bufs=6, space="PSUM"))
        psB = ctx.enter_context(tc.tile_pool(name="psB", bufs=2, space="PSUM"))

        gf_sb = big.tile([P, NCT, HW], F16, tag="gf")
        lf_sb = big.tile([P, NCT, HW], F16, tag="lf")
        u_sb = big.tile([P, NCT, HW], F16, tag="u")
        at_sb = small.tile([P, NCT, NCT, P], F16, tag="at")
        vecs_sb = small.tile([P, NCT, 3], F16, tag="vecs")
        wtile = small.tile([P, P], F16, tag="warm")
        clf_row = small.tile([1, HW], F32, tag="clf")    # convlf, row space

        r_sb = small.tile([P, NKT], F32, tag="r")
        vwg32 = small.tile([P, NKT], F32, tag="vwg")
        biasR = small.tile([P, NKT], F32, tag="biasR")
        vwones = small.tile([P, 2, NKT], BF16, tag="vwones")

        # ---- warm-up: memset a scratch tile, then NWARM dummy matmuls so the
        # PE HAM clock-gate reaches 8/8 (2.4GHz) during the initial DMA wait.
        nc.gpsimd.memset(wtile, 0.015625)
        wps = psB.tile([P, P], F32, tag="nd")
        for _ in range(NWARM):
            nc.tensor.matmul(wps, wtile, wtile, start=True, stop=True)

        nc.vector.memset(vwones[:, 1:2, :], 1.0)

        # ---- input DMAs (v1's proven fine-grained schedule): tiny 256-col
        # first slices so the first matmuls unblock fast, then 1024-col
        # slices; alternate queues so transfers parallelize.
        nc.sync.dma_start(at_sb[:, 0:1], at_d[:, 0:1])
        nc.sync.dma_start(at_sb[:, 1:4], at_d[:, 1:4])
        nc.gpsimd.dma_start(vecs_sb, vecs_d)
        SLICES = [(0, 256), (256, 1024), (1280, 1024)]
        for si, (h0, hw_) in enumerate(SLICES):
            for t in range(NCT):
                if si == 0:
                    eng = nc.scalar
                else:
                    eng = nc.sync if t % 2 == 0 else nc.scalar
                eng.dma_start(gf_sb[:, t, h0 : h0 + hw_], gf_d[t][:, h0 : h0 + hw_])
        for si, (h0, hw_) in enumerate(SLICES):
            for t in range(NCT):
                eng = (nc.gpsimd, nc.sync, nc.scalar)[(si * NCT + t) % 3]
                eng.dma_start(lf_sb[:, t, h0 : h0 + hw_], lf_d[t][:, h0 : h0 + hw_])

        # ---- phase 1 per chunk: 1b (rowterm/vw.gf) FIRST so the reshape
        # round-trip overlaps the rest of phase 1, then 1a (U = A gf).
        vec_stores = []
        for ci_, (q0, w) in enumerate(CHUNKS):
            ps2 = psB.tile([2, w], F32, tag="nd")
            for ci in range(NCT):
                nc.tensor.matmul(
                    ps2,
                    vecs_sb[:, ci, 0:2],
                    gf_sb[:, ci, q0 : q0 + w],
                    start=(ci == 0),
                    stop=(ci == NCT - 1),
                )
            st = stg.tile([2, w], F32, tag="vstage")
            nc.scalar.copy(st, ps2)
            eng = nc.sync if ci_ % 2 == 0 else nc.gpsimd
            vec_stores.append(eng.dma_start(vtmp[:, q0 : q0 + w], st))

            for co in range(NCT):
                ps = psA.tile([P, w], F32, tag="ps")
                for ci in range(NCT):
                    nc.tensor.matmul(
                        ps,
                        at_sb[:, co, ci, :],
                        gf_sb[:, ci, q0 : q0 + w],
                        start=(ci == 0),
                        stop=(ci == NCT - 1),
                    )
                nc.scalar.copy(u_sb[:, co, q0 : q0 + w], ps)

        # ---- reshape rowterm / vw.gf into [128,18] partition-major tiles
        # (q = t*128 + p bijection) and build the per-key exp bias.
        ld = nc.sync.dma_start(r_sb, vtmp[0].rearrange("(t p) -> p t", p=P))
        for s in vec_stores:
            add_dep_helper(ld.ins, s.ins, reason="dram raw rowterm")
        ld = nc.gpsimd.dma_start(vwg32, vtmp[1].rearrange("(t p) -> p t", p=P))
        for s in vec_stores:
            add_dep_helper(ld.ins, s.ins, reason="dram raw vwgf")
        nc.vector.tensor_scalar_add(biasR, r_sb, -CM)
        nc.vector.tensor_copy(vwones[:, 0:1, :], vwg32)

        # ---- phase 1c: convlf = Wconv . lf -> clf_row (stays in row space)
        for ci_, (q0, w) in enumerate(CHUNKS):
            ps3 = psB.tile([2, w], F32, tag="nd")
            for ci in range(NCT):
                nc.tensor.matmul(
                    ps3[0:1, :],
                    vecs_sb[:, ci, 2:3],
                    lf_sb[:, ci, q0 : q0 + w],
                    start=(ci == 0),
                    stop=(ci == NCT - 1),
                )
            nc.scalar.copy(clf_row[0:1, q0 : q0 + w], ps3[0:1, :])

        # ---- phase 2 per chunk: logits + exp for all 18 k-tiles, then the 18
        # num/den matmuls back-to-back (batching bf16 after fp16 avoids the
        # ~95ns PE dtype-switch penalty at every tile boundary).  Division +
        # convlf add happen in row space; result DMAs straight to out[q0:].
        for ci_, (q0, w) in enumerate(CHUNKS):
            pexps = []
            for kt in range(NKT):
                t0 = psA.tile([P, w], F32, tag="ps")
                for ct in range(NCT):
                    nc.tensor.matmul(
                        t0,
                        u_sb[:, ct, kt * P : (kt + 1) * P],
                        lf_sb[:, ct, q0 : q0 + w],
                        start=(ct == 0),
                        stop=(ct == NCT - 1),
                    )
                pexp = ppool.tile([P, w], BF16, tag="pexp")
                nc.scalar.activation(
                    pexp, t0, _EXP, bias=biasR[:, kt : kt + 1], scale=1.0
                )
                pexps.append(pexp)

            nd = psB.tile([2, w], F32, tag="nd")
            for kt in range(NKT):
                nc.tensor.matmul(
                    nd,
                    vwones[:, :, kt : kt + 1],
                    pexps[kt],
                    start=(kt == 0),
                    stop=(kt == NKT - 1),
                )

            nd2 = rows.tile([2, w], F32, tag="nd2")
            nc.vector.tensor_copy(nd2, nd)
            # engines need 32-aligned partition bases, so the den row
            # (partition 1) moves to partition 0 of its own tile via a DRAM
            # bounce (the runtime rejects SBUF->SBUF descriptors here).
            den0 = rows.tile([1, w], F32, tag="den0")
            deng = nc.sync if ci_ % 2 == 0 else nc.gpsimd
            dst_ = deng.dma_start(dtmp[0:1, q0 : q0 + w], nd2[1:2, :])
            dld = deng.dma_start(den0, dtmp[0:1, q0 : q0 + w])
            add_dep_helper(dld.ins, dst_.ins, reason="dram raw den")
            rec = rows.tile([1, w], F32, tag="rec")
            nc.vector.reciprocal(rec, den0)
            res = rows.tile([1, w], F32, tag="res")
            nc.vector.tensor_mul(res, nd2[0:1, :], rec)
            fin = rows.tile([1, w], F32, tag="fin")
            nc.vector.scalar_tensor_tensor(
                fin, res, float(const_add), clf_row[0:1, q0 : q0 + w],
                op0=_ADD, op1=_ADD,
            )
            # NOTE: fin (2-D [1,w]) not fin[0] — a fully-1-D SBUF source AP
            # makes the runtime reject the NEFF at load time.
            deng.dma_start(out_d[q0 : q0 + w], fin)

    nc.compile()
    return nc


_CACHE: dict[bytes, bacc.Bacc] = {}


def _fold(inputs):
    f64 = np.float64
    Wq, bq = inputs["Wq"].astype(f64), inputs["bq"].astype(f64)
    Wk, bk = inputs["Wk"].astype(f64), inputs["bk"].astype(f64)
    Wv, bv = inputs["Wv"].astype(f64), inputs["bv"].astype(f64)
    Wo, bo = inputs["Wo"].astype(f64), inputs["bo"].astype(f64)
    Wconv, bconv = inputs["Wconv"].astype(f64), inputs["bconv"].astype(f64)

    A = Wq.T @ Wk                       # S0 = lf^T A gf
    AT = np.ascontiguousarray(
        A.T.astype(np.float16).reshape(NCT, P, NCT, P).transpose(1, 2, 0, 3)
    )
    wkb = Wk.T @ bq                     # rowterm = wkb^T gf
    weff = Wo.T @ Wconv[0]
    wv = Wv.T @ weff
    vecs = np.stack(
        [wkb.astype(np.float32), wv.astype(np.float32), inputs["Wconv"][0]], axis=1
    )                                   # [C, 3]
    vecs = np.ascontiguousarray(
        vecs.astype(np.float16).reshape(NCT, P, 3).transpose(1, 0, 2)
    )
    const_add = float(weff @ bv + Wconv[0] @ bo + bconv[0])
    return AT, vecs, const_add


def _prepare_in_maps(inputs):
    AT, vecs, const_add = _fold(inputs)
    lf = np.ascontiguousarray(inputs["local_feat"].astype(np.float16)).reshape(
        NCORES, NCT, P, HW
    )
    gf = np.ascontiguousarray(inputs["global_feat"].astype(np.float16)).reshape(
        NCORES, NCT, P, HW
    )
    in_maps = [
        {"lf": lf[b], "gf": gf[b], "at": AT, "vecs": vecs} for b in range(NCORES)
    ]
    return in_maps, const_add


def run(inputs, trace: bool = False, **kwargs):
    """Run on hardware; returns (output [8,1,48,48], BassKernelResults)."""
    in_maps, const_add = _prepare_in_maps(inputs)
    key = np.float32(const_add).tobytes()
    if key not in _CACHE:
        _CACHE[key] = _build_program(const_add)
    nc = _CACHE[key]
    res = run_bass_kernel_spmd(
        nc, in_maps, core_ids=list(range(NCORES)), trace=trace, **kwargs
    )
    out = np.stack([res.results[b]["out"] for b in range(NCORES)], axis=0)
    return out.reshape(NCORES, 1, 48, 48).astype(np.float32), res


def kernel(**inputs) -> np.ndarray:
    out, _ = run(inputs)
    return out
